# revision 29
# baseline (speedup 1.0000x reference)
"""DeepSeekMoE layer on 8 Trainium2 NeuronCores.

Strategy (expert-parallel):
  - Host: RMSNorm + router matmul + top-k + per-expert token gather
    (routing is tiny: 2048x1024 @ 1024x64). Tokens for each routed
    expert are gathered into per-expert slots and bin-packed onto the
    8 cores (rank-grouped so slot j holds similarly-sized experts on
    every core; per-slot capacities are compile-time constants rounded
    to 32).
  - Device (SPMD, one program on cores 0-7): for each expert slot,
    stream W1/W2 from HBM once and run the FFN on the gathered token
    batch entirely transposed ([D, tokens] layout) so every matmul
    contracts along partitions:
        h^T = W1^T x^T (+b1); g^T = silu(Wg^T h^T) * h^T;
        o^T = W2^T g^T (+b2)
    Default precision: bf16 weights/activations with fp32 PSUM
    accumulate (~3.6e-4 final rel err, ~30MB HBM traffic per core).
    PREC="f32r" switches to fp32 words with TF32-like matmuls
    (~2e-5 rel err, ~61MB per core). The two shared experts run as a
    10th slot (each core does 512 tokens of one shared expert). All
    arrays are pre-arranged on the host into SBUF layout
    ([128, k-chunks, tokens]) so every DMA is a contiguous 2D copy at
    HBM line rate. Weight loads issue on the sync HWDGE ring, token
    loads on the scalar ring; output stores also ride the sync ring but
    are emitted two sections late so a store (which waits on compute)
    never head-of-line-blocks weight prefetch. Section 0 loads W1 as 4
    m-tiles and tokens as 8 k-tiles to cut the cold-start head; the
    final section stores per-chunk to cut the tail. PSUM tiles hold
    pairs of m-tiles so PSUM->SBUF moves are single fat DVE ops when
    biases are zero.
  - Host: scatter-add weighted expert outputs + shared + residual.

Self-contained: shapes hardcoded for B=2, S=1024, D=1024, H=512,
E_R=64, K=6, E_S=2.
"""

import numpy as np
from contextlib import ExitStack

B, S, D, H, E_R, K, E_S = 2, 1024, 1024, 512, 64, 6, 2
T = B * S
EPS = 1.1920929e-07

PREC = "bf16"        # "bf16" (~3.6e-4 rel err) or "f32r" (~2e-5 rel err, 2x traffic)
N_SLOTS = 8          # routed expert slots per core
SH_TOK = 512         # shared-expert tokens per core
KD = D // 128        # 8 k-tiles for the D contraction
KH = H // 128        # 4 k-tiles for the H contraction

_PROG_CACHE = {}


def _np_wdt():
    if PREC == "bf16":
        import ml_dtypes
        return ml_dtypes.bfloat16
    return np.float32


def _prearrange(w, ktiles):
    """[K*128, N] -> [128, K*N] so each SBUF partition's row is one
    contiguous DRAM read."""
    n = w.shape[1]
    return np.ascontiguousarray(
        w.reshape(ktiles, 128, n).transpose(1, 0, 2).reshape(128, ktiles * n)
    ).astype(_np_wdt())


def _prearrange_w1(w):
    """[D, H] -> [128, KH*KD*128] m-major ([m][k][i] per partition) so W1 can
    load as 4 independent m-tiles."""
    return np.ascontiguousarray(
        w.reshape(KD, 128, KH, 128).transpose(1, 2, 0, 3).reshape(128, KH * KD * 128)
    ).astype(_np_wdt())


def _build_program(caps, has_bias):
    """caps: per-section token capacities; the last entry is the shared
    512-token section, the rest are routed expert slots."""
    import concourse.tile as tile
    from concourse import bacc, mybir

    f32 = mybir.dt.float32
    wdt = mybir.dt.float32r if PREC == "f32r" else mybir.dt.bfloat16
    odt = f32 if PREC == "f32r" else mybir.dt.bfloat16
    AF = mybir.ActivationFunctionType

    n_slots = len(caps) - 1
    offs = np.concatenate([[0], np.cumsum(caps)])
    sumcap = int(offs[-2])          # routed columns only
    sh_cap = caps[-1]

    nc = bacc.Bacc("TRN2", target_bir_lowering=False, debug=False)

    xgt = nc.dram_tensor("xgt", [128, KD * sumcap], wdt, kind="ExternalInput").ap()
    w1s = nc.dram_tensor("w1s", [n_slots, 128, KD * H], wdt, kind="ExternalInput").ap()
    w2s = nc.dram_tensor("w2s", [n_slots, 128, KH * D], wdt, kind="ExternalInput").ap()
    b1s = nc.dram_tensor("b1s", [128, n_slots * 4], f32, kind="ExternalInput").ap()
    b2s = nc.dram_tensor("b2s", [128, n_slots * 8], f32, kind="ExternalInput").ap()
    wg = nc.dram_tensor("wg", [128, KH * H], wdt, kind="ExternalInput").ap()
    swg = nc.dram_tensor("swg", [128, KH * H], wdt, kind="ExternalInput").ap()
    tsht = nc.dram_tensor("tsht", [128, KD * sh_cap], wdt, kind="ExternalInput").ap()
    sw1 = nc.dram_tensor("sw1", [128, KD * H], wdt, kind="ExternalInput").ap()
    sw2 = nc.dram_tensor("sw2", [128, KH * D], wdt, kind="ExternalInput").ap()
    sb1 = nc.dram_tensor("sb1", [128, 4], f32, kind="ExternalInput").ap()
    sb2 = nc.dram_tensor("sb2", [128, 8], f32, kind="ExternalInput").ap()
    rout = nc.dram_tensor("rout", [128, KD * sumcap], odt, kind="ExternalOutput").ap()
    sout = nc.dram_tensor("sout", [128, KD * sh_cap], odt, kind="ExternalOutput").ap()

    with tile.TileContext(nc) as tc:
        with ExitStack() as ctx:
            consts = ctx.enter_context(tc.tile_pool(name="consts", bufs=1))
            w1p = ctx.enter_context(tc.tile_pool(name="w1p", bufs=5))
            w2p = ctx.enter_context(tc.tile_pool(name="w2p", bufs=5))
            xgp = ctx.enter_context(tc.tile_pool(name="xgp", bufs=4))
            hp = ctx.enter_context(tc.tile_pool(name="hp", bufs=3))
            gpp = ctx.enter_context(tc.tile_pool(name="gpp", bufs=2))
            gp = ctx.enter_context(tc.tile_pool(name="gp", bufs=3))
            op = ctx.enter_context(tc.tile_pool(name="op", bufs=3))
            php = ctx.enter_context(tc.tile_pool(name="php", bufs=2, space="PSUM"))
            pgp = ctx.enter_context(tc.tile_pool(name="pgp", bufs=2, space="PSUM"))
            pop = ctx.enter_context(tc.tile_pool(name="pop", bufs=4, space="PSUM"))

            # biases ride the idle gpsimd ring so the sync ring's first
            # issue is section 0's W1; gate weights ride the scalar ring
            # right after the first token load (see after_xg below)
            b1_sb = consts.tile([128, n_slots * 4], f32, tag="b1")
            nc.gpsimd.dma_start(b1_sb[:], b1s[:])
            b2_sb = consts.tile([128, n_slots * 8], f32, tag="b2")
            nc.gpsimd.dma_start(b2_sb[:], b2s[:])
            sb1_sb = consts.tile([128, 4], f32, tag="sb1")
            nc.gpsimd.dma_start(sb1_sb[:], sb1[:])
            sb2_sb = consts.tile([128, 8], f32, tag="sb2")
            nc.gpsimd.dma_start(sb2_sb[:], sb2[:])
            wg_sb = consts.tile([128, KH * H], wdt, tag="wg")
            swg_sb = consts.tile([128, KH * H], wdt, tag="swg")

            def section(cap, w1_src, w2_src, xg_src, out_dst, wgt, b1ap, b2ap,
                        after_xg=None, last=False, w1_tiles=None, xg_tiles=None,
                        after_loads=None):
                """One expert FFN pass over `cap` tokens (transposed layout).
                PSUM tiles hold pairs of m-tiles (one bank) so PSUM->SBUF
                moves are single fat ops when biases are zero."""
                p2 = 2 if cap <= 256 else 1   # m-tiles per PSUM bank
                if xg_tiles is None:
                    xgsb = xgp.tile([128, KD * cap], wdt, tag="xg")
                    nc.scalar.dma_start(xgsb[:], xg_src)
                if after_xg is not None:
                    after_xg()
                if w1_tiles is None:
                    w1sb = w1p.tile([128, KD * H], wdt, tag="w1")
                    nc.sync.dma_start(w1sb[:], w1_src)
                w2sb = w2p.tile([128, KH * D], wdt, tag="w2")
                nc.sync.dma_start(w2sb[:], w2_src)
                if after_loads is not None:
                    after_loads()

                def w1ap(m, k):
                    if w1_tiles is not None:
                        return w1_tiles[m][:, k * 128 : (k + 1) * 128]
                    return w1sb[:, m * KD * 128 + k * 128 : m * KD * 128 + (k + 1) * 128]

                def xgap(k):
                    if xg_tiles is not None:
                        return xg_tiles[k][:]
                    return xgsb[:, k * cap : (k + 1) * cap]

                hsb = hp.tile([128, KH * cap], wdt, tag="h")
                for mp in range(KH // p2):
                    ph = php.tile([128, p2 * cap], f32, tag="ph")
                    for sub in range(p2):
                        m = p2 * mp + sub
                        for k in range(KD):
                            nc.tensor.matmul(
                                ph[:, sub * cap : (sub + 1) * cap],
                                w1ap(m, k),
                                xgap(k),
                                start=(k == 0),
                                stop=(k == KD - 1),
                            )
                    if has_bias:
                        for sub in range(p2):
                            m = p2 * mp + sub
                            nc.vector.tensor_scalar_add(
                                hsb[:, m * cap : (m + 1) * cap],
                                ph[:, sub * cap : (sub + 1) * cap],
                                b1ap[:, m : m + 1],
                            )
                    else:
                        nc.vector.tensor_copy(
                            hsb[:, p2 * mp * cap : p2 * (mp + 1) * cap], ph[:]
                        )

                gsb = gp.tile([128, KH * cap], wdt, tag="g")
                for mp in range(KH // p2):
                    pg = pgp.tile([128, p2 * cap], f32, tag="pg")
                    for sub in range(p2):
                        m = p2 * mp + sub
                        for k in range(KH):
                            nc.tensor.matmul(
                                pg[:, sub * cap : (sub + 1) * cap],
                                wgt[:, k * H + m * 128 : k * H + (m + 1) * 128],
                                hsb[:, k * cap : (k + 1) * cap],
                                start=(k == 0),
                                stop=(k == KH - 1),
                            )
                    gpre = gpp.tile([128, p2 * cap], wdt, tag="gpre")
                    nc.scalar.activation(gpre[:], pg[:], AF.Silu)
                    nc.vector.tensor_mul(
                        gsb[:, p2 * mp * cap : p2 * (mp + 1) * cap],
                        gpre[:],
                        hsb[:, p2 * mp * cap : p2 * (mp + 1) * cap],
                    )

                osb = op.tile([128, KD * cap], odt, tag="o")
                for mp in range(KD // p2):
                    po = pop.tile([128, p2 * cap], f32, tag="po")
                    for sub in range(p2):
                        m = p2 * mp + sub
                        for k in range(KH):
                            nc.tensor.matmul(
                                po[:, sub * cap : (sub + 1) * cap],
                                w2sb[:, k * D + m * 128 : k * D + (m + 1) * 128],
                                gsb[:, k * cap : (k + 1) * cap],
                                start=(k == 0),
                                stop=(k == KH - 1),
                            )
                    if has_bias:
                        for sub in range(p2):
                            m = p2 * mp + sub
                            nc.vector.tensor_scalar_add(
                                osb[:, m * cap : (m + 1) * cap],
                                po[:, sub * cap : (sub + 1) * cap],
                                b2ap[:, m : m + 1],
                            )
                    else:
                        nc.vector.tensor_copy(
                            osb[:, p2 * mp * cap : p2 * (mp + 1) * cap], po[:]
                        )
                    if last:
                        # final section: store chunks immediately so the
                        # kernel tail is one chunk, not 2MB
                        nc.sync.dma_start(
                            out_dst[:, p2 * mp * cap : p2 * (mp + 1) * cap],
                            osb[:, p2 * mp * cap : p2 * (mp + 1) * cap],
                        )
                if last:
                    return None
                def store():
                    nc.sync.dma_start(out_dst, osb[:])
                return store

            def load_wg():
                nc.scalar.dma_start(wg_sb[:], wg[:])

            def load_swg():
                nc.scalar.dma_start(swg_sb[:], swg[:])

            # section 0 fast start: W1 as 4 independent m-tiles and tokens as
            # 8 k-tiles so the first matmul waits for ~600KB, not 3MB
            c0 = caps[0]
            sec0_xg = [consts.tile([128, c0], wdt, tag=f"xh{k}", name=f"xh{k}") for k in range(KD)]
            nc.scalar.dma_start(sec0_xg[0][:], xgt[:, 0:c0])
            nc.scalar.dma_start(sec0_xg[1][:], xgt[:, c0 : 2 * c0])
            load_wg()
            for k in range(2, KD):
                nc.scalar.dma_start(sec0_xg[k][:], xgt[:, k * c0 : (k + 1) * c0])
            sec0_w1 = [consts.tile([128, KD * 128], wdt, tag=f"w1h{m}", name=f"w1h{m}") for m in range(KH)]
            for m in range(KH):
                nc.sync.dma_start(
                    sec0_w1[m][:], w1s[0][:, m * KD * 128 : (m + 1) * KD * 128]
                )

            pending = []

            def flush_store():
                if len(pending) >= 2:
                    pending.pop(0)()

            for j in range(n_slots):
                lo, hi = int(offs[j]) * KD, int(offs[j + 1]) * KD
                st = section(
                    caps[j],
                    w1s[j],
                    w2s[j],
                    xgt[:, lo:hi],
                    rout[:, lo:hi],
                    wg_sb,
                    b1_sb[:, j * 4 : (j + 1) * 4],
                    b2_sb[:, j * 8 : (j + 1) * 8],
                    after_xg=load_swg if j == 1 else None,
                    w1_tiles=sec0_w1 if j == 0 else None,
                    xg_tiles=sec0_xg if j == 0 else None,
                    after_loads=flush_store,
                )
                pending.append(st)

            section(sh_cap, sw1[:], sw2[:], tsht[:], sout[:], swg_sb, sb1_sb,
                    sb2_sb, last=True, after_loads=flush_store)
            for st in pending:
                st()

    nc.compile()
    return nc


def _get_program(caps, has_bias):
    key = (tuple(caps), PREC, has_bias)
    if key not in _PROG_CACHE:
        _PROG_CACHE[key] = _build_program(tuple(caps), has_bias)
    return _PROG_CACHE[key]


def _route(x, norm_w, Wr, bias):
    """Host-side norm + router + top-k (matches jax.lax.top_k tie-breaking)."""
    xf = x.reshape(T, D).astype(np.float32)
    ms = np.mean(xf * xf, axis=-1, keepdims=True, dtype=np.float32)
    t = (xf * (1.0 / np.sqrt(ms + EPS)) * norm_w).astype(np.float32)
    raw = t @ Wr.T
    aff = raw + bias
    idx = np.argsort(-aff, axis=-1, kind="stable")[:, :K]
    aff_k = np.take_along_axis(raw, idx, axis=1)
    w = aff_k / aff_k.sum(-1, keepdims=True)
    return t, idx.astype(np.int64), w.astype(np.float32)


def _gather_block(t, toks, cap):
    """tokens (cnt, D) -> [128, KD, cap] SBUF layout block (zero padded)."""
    blk = np.zeros((128, KD, cap), _np_wdt())
    g = t[toks].T.reshape(KD, 128, len(toks)).transpose(1, 0, 2)
    blk[:, :, : len(toks)] = g
    return blk


def _decode_block(blk, cnt):
    """[128, KD, cap] device output block -> (cnt, D) token outputs."""
    cap = blk.shape[2]
    return blk.transpose(1, 0, 2).reshape(D, cap)[:, :cnt].T.astype(np.float32)


def kernel(**inputs):
    x = np.asarray(inputs["x"], dtype=np.float32)
    norm_w = np.asarray(inputs["norm_w"], dtype=np.float32)
    Wr = np.asarray(inputs["Wr"], dtype=np.float32)
    bias = np.asarray(inputs["bias"], dtype=np.float32)
    sW1 = np.asarray(inputs["sW1"], dtype=np.float32)
    sb1 = np.asarray(inputs["sb1"], dtype=np.float32)
    sW2 = np.asarray(inputs["sW2"], dtype=np.float32)
    sb2 = np.asarray(inputs["sb2"], dtype=np.float32)
    sWg = np.asarray(inputs["sWg"], dtype=np.float32)
    rW1 = np.asarray(inputs["rW1"], dtype=np.float32)
    rb1 = np.asarray(inputs["rb1"], dtype=np.float32)
    rW2 = np.asarray(inputs["rW2"], dtype=np.float32)
    rb2 = np.asarray(inputs["rb2"], dtype=np.float32)
    rWg = np.asarray(inputs["rWg"], dtype=np.float32)

    t, idx, w = _route(x, norm_w, Wr, bias)

    # per-expert token lists (token order ascending within each expert)
    flat_e = idx.ravel()
    flat_tok = np.repeat(np.arange(T), K)
    flat_w = w.ravel()
    order = np.argsort(flat_e, kind="stable")
    se, st, sw = flat_e[order], flat_tok[order], flat_w[order]
    counts = np.bincount(flat_e, minlength=E_R)
    bounds = np.concatenate([[0], np.cumsum(counts)])

    # split any over-512 expert into <=512 pieces (512 = max matmul free dim
    # for one PSUM bank at fp32)
    pieces = []  # (expert, tok_ids, weights)
    for e in range(E_R):
        lo, hi = bounds[e], bounds[e + 1]
        for s in range(lo, hi, 512):
            pieces.append((e, st[s : min(s + 512, hi)], sw[s : min(s + 512, hi)]))
    n_slots = max(N_SLOTS, -(-len(pieces) // 8))

    # snake assignment: sort pieces by size desc; rank-group of 8 -> one slot
    # index across all cores; within each group assign large->small to the
    # cores with the smallest running totals. Slot capacity = group max
    # rounded up to 16 (compile-time constant; identical inputs -> identical
    # caps -> NEFF cache hit).
    pieces.sort(key=lambda p: -len(p[1]))
    slot_of_core = [[None] * n_slots for _ in range(8)]
    totals = np.zeros(8, np.int64)
    caps = []
    for j in range(n_slots):
        group = pieces[j * 8 : (j + 1) * 8]
        core_order = np.argsort(totals, kind="stable")
        for gi, piece in enumerate(group):
            c = core_order[gi]
            slot_of_core[c][j] = piece
            totals[c] += len(piece[1])
        gmax = max((len(p[1]) for p in group), default=16)
        caps.append(min(512, max(32, -(-gmax // 16) * 16)))
    if PREC == "f32r":
        caps = [max(256, c) for c in caps]  # f32r needs N>=256 for full rate
    caps.append(SH_TOK)

    has_bias = bool(
        np.any(rb1) or np.any(rb2) or np.any(sb1) or np.any(sb2)
    )
    nc = _get_program(caps, has_bias)
    offs = np.concatenate([[0], np.cumsum(caps)]).astype(int)
    sumcap = int(offs[-2])

    wg_pre = _prearrange(rWg, KH)
    swg_pre = _prearrange(sWg, KH)
    sw1_pre = [_prearrange_w1(sW1[e]) for e in range(E_S)]
    sw2_pre = [_prearrange(sW2[e], KH) for e in range(E_S)]
    w1_pre = {}
    w2_pre = {}
    in_maps = []
    for c in range(8):
        xgt = np.zeros((128, KD * sumcap), _np_wdt())
        w1_stack = np.zeros((n_slots, 128, KD * H), _np_wdt())
        w2_stack = np.zeros((n_slots, 128, KH * D), _np_wdt())
        b1_arr = np.zeros((128, n_slots * 4), np.float32)
        b2_arr = np.zeros((128, n_slots * 8), np.float32)
        for j in range(n_slots):
            piece = slot_of_core[c][j]
            if piece is None:
                continue
            e, toks, _ = piece
            xgt[:, offs[j] * KD : offs[j + 1] * KD] = _gather_block(
                t, toks, caps[j]
            ).reshape(128, KD * caps[j])
            if e not in w1_pre:
                w1_pre[e] = _prearrange_w1(rW1[e])
                w2_pre[e] = _prearrange(rW2[e], KH)
            w1_stack[j] = w1_pre[e]
            w2_stack[j] = w2_pre[e]
            b1_arr[:, j * 4 : (j + 1) * 4] = rb1[e, 0].reshape(4, 128).T
            b2_arr[:, j * 8 : (j + 1) * 8] = rb2[e, 0].reshape(8, 128).T
        qc, se_ = c % 4, c // 4
        sh_toks = np.arange(qc * SH_TOK, (qc + 1) * SH_TOK)
        in_maps.append({
            "xgt": xgt,
            "w1s": w1_stack,
            "w2s": w2_stack,
            "b1s": b1_arr,
            "b2s": b2_arr,
            "wg": wg_pre,
            "swg": swg_pre,
            "tsht": _gather_block(t, sh_toks, SH_TOK).reshape(128, KD * SH_TOK),
            "sw1": sw1_pre[se_],
            "sw2": sw2_pre[se_],
            "sb1": sb1[se_, 0].reshape(4, 128).T.copy(),
            "sb2": sb2[se_, 0].reshape(8, 128).T.copy(),
        })

    from concourse.bass_utils import run_bass_kernel_spmd

    global _LAST_IN_MAPS
    _LAST_IN_MAPS = in_maps
    res = run_bass_kernel_spmd(nc, in_maps, core_ids=list(range(8)))

    out = x.reshape(T, D).copy()
    for c in range(8):
        qc = c % 4
        so = res.results[c]["sout"].reshape(128, KD, SH_TOK)
        out[qc * SH_TOK : (qc + 1) * SH_TOK] += _decode_block(so, SH_TOK)
        ro = res.results[c]["rout"]
        for j in range(n_slots):
            piece = slot_of_core[c][j]
            if piece is None:
                continue
            _, toks, wv = piece
            blk = ro[:, offs[j] * KD : offs[j + 1] * KD].reshape(128, KD, caps[j])
            out[toks] += wv[:, None] * _decode_block(blk, len(toks))
    return out.reshape(B, S, D).astype(np.float32)


# revision 30
# speedup vs baseline: 1.0199x; 1.0199x over previous
"""DeepSeekMoE layer on 8 Trainium2 NeuronCores.

Strategy (expert-parallel):
  - Host: RMSNorm + router matmul + top-k + per-expert token gather
    (routing is tiny: 2048x1024 @ 1024x64). Tokens for each routed
    expert are gathered into per-expert slots and bin-packed onto the
    8 cores (rank-grouped so slot j holds similarly-sized experts on
    every core; per-slot capacities are compile-time constants rounded
    to 32).
  - Device (SPMD, one program on cores 0-7): for each expert slot,
    stream W1/W2 from HBM once and run the FFN on the gathered token
    batch entirely transposed ([D, tokens] layout) so every matmul
    contracts along partitions:
        h^T = W1^T x^T (+b1); g^T = silu(Wg^T h^T) * h^T;
        o^T = W2^T g^T (+b2)
    Default precision: bf16 weights/activations with fp32 PSUM
    accumulate (~3.6e-4 final rel err, ~30MB HBM traffic per core).
    PREC="f32r" switches to fp32 words with TF32-like matmuls
    (~2e-5 rel err, ~61MB per core). The two shared experts run as a
    10th slot (each core does 512 tokens of one shared expert). All
    arrays are pre-arranged on the host into SBUF layout
    ([128, k-chunks, tokens]) so every DMA is a contiguous 2D copy at
    HBM line rate. Weight loads issue on the sync HWDGE ring, token
    loads on the scalar ring; output stores also ride the sync ring but
    are emitted two sections late so a store (which waits on compute)
    never head-of-line-blocks weight prefetch. Section 0 loads W1 as 4
    m-tiles and tokens as 8 k-tiles to cut the cold-start head; the
    final section stores per-chunk to cut the tail. PSUM tiles hold
    pairs of m-tiles so PSUM->SBUF moves are single fat DVE ops when
    biases are zero.
  - Host: scatter-add weighted expert outputs + shared + residual.

Self-contained: shapes hardcoded for B=2, S=1024, D=1024, H=512,
E_R=64, K=6, E_S=2.
"""

import numpy as np
from contextlib import ExitStack

B, S, D, H, E_R, K, E_S = 2, 1024, 1024, 512, 64, 6, 2
T = B * S
EPS = 1.1920929e-07

PREC = "bf16"        # "bf16" (~3.6e-4 rel err) or "f32r" (~2e-5 rel err, 2x traffic)
N_SLOTS = 8          # routed expert slots per core
SH_TOK = 512         # shared-expert tokens per core
KD = D // 128        # 8 k-tiles for the D contraction
KH = H // 128        # 4 k-tiles for the H contraction

_PROG_CACHE = {}


def _np_wdt():
    if PREC == "bf16":
        import ml_dtypes
        return ml_dtypes.bfloat16
    return np.float32


def _prearrange(w, ktiles):
    """[K*128, N] -> [128, K*N] so each SBUF partition's row is one
    contiguous DRAM read."""
    n = w.shape[1]
    return np.ascontiguousarray(
        w.reshape(ktiles, 128, n).transpose(1, 0, 2).reshape(128, ktiles * n)
    ).astype(_np_wdt())


def _prearrange_w1(w):
    """[D, H] -> [128, KH*KD*128] m-major ([m][k][i] per partition) so W1 can
    load as 4 independent m-tiles."""
    return np.ascontiguousarray(
        w.reshape(KD, 128, KH, 128).transpose(1, 2, 0, 3).reshape(128, KH * KD * 128)
    ).astype(_np_wdt())


def _build_program(caps, has_bias):
    """caps: per-section token capacities; the last entry is the shared
    512-token section, the rest are routed expert slots."""
    import concourse.tile as tile
    from concourse import bacc, mybir

    f32 = mybir.dt.float32
    wdt = mybir.dt.float32r if PREC == "f32r" else mybir.dt.bfloat16
    odt = f32 if PREC == "f32r" else mybir.dt.bfloat16
    AF = mybir.ActivationFunctionType

    n_slots = len(caps) - 1
    offs = np.concatenate([[0], np.cumsum(caps)])
    sumcap = int(offs[-2])          # routed columns only
    sh_cap = caps[-1]

    nc = bacc.Bacc("TRN2", target_bir_lowering=False, debug=False)

    xgt = nc.dram_tensor("xgt", [128, KD * sumcap], wdt, kind="ExternalInput").ap()
    w1s = nc.dram_tensor("w1s", [n_slots, 128, KD * H], wdt, kind="ExternalInput").ap()
    w2s = nc.dram_tensor("w2s", [n_slots, 128, KH * D], wdt, kind="ExternalInput").ap()
    b1s = nc.dram_tensor("b1s", [128, n_slots * 4], f32, kind="ExternalInput").ap()
    b2s = nc.dram_tensor("b2s", [128, n_slots * 8], f32, kind="ExternalInput").ap()
    wg = nc.dram_tensor("wg", [128, KH * H], wdt, kind="ExternalInput").ap()
    swg = nc.dram_tensor("swg", [128, KH * H], wdt, kind="ExternalInput").ap()
    tsht = nc.dram_tensor("tsht", [128, KD * sh_cap], wdt, kind="ExternalInput").ap()
    sw1 = nc.dram_tensor("sw1", [128, KD * H], wdt, kind="ExternalInput").ap()
    sw2 = nc.dram_tensor("sw2", [128, KH * D], wdt, kind="ExternalInput").ap()
    sb1 = nc.dram_tensor("sb1", [128, 4], f32, kind="ExternalInput").ap()
    sb2 = nc.dram_tensor("sb2", [128, 8], f32, kind="ExternalInput").ap()
    rout = nc.dram_tensor("rout", [128, KD * sumcap], odt, kind="ExternalOutput").ap()
    sout = nc.dram_tensor("sout", [128, KD * sh_cap], odt, kind="ExternalOutput").ap()

    with tile.TileContext(nc) as tc:
        with ExitStack() as ctx:
            consts = ctx.enter_context(tc.tile_pool(name="consts", bufs=1))
            w1p = ctx.enter_context(tc.tile_pool(name="w1p", bufs=5))
            w2p = ctx.enter_context(tc.tile_pool(name="w2p", bufs=5))
            xgp = ctx.enter_context(tc.tile_pool(name="xgp", bufs=4))
            hp = ctx.enter_context(tc.tile_pool(name="hp", bufs=3))
            gpp = ctx.enter_context(tc.tile_pool(name="gpp", bufs=2))
            gp = ctx.enter_context(tc.tile_pool(name="gp", bufs=3))
            op = ctx.enter_context(tc.tile_pool(name="op", bufs=3))
            php = ctx.enter_context(tc.tile_pool(name="php", bufs=3, space="PSUM"))
            pgp = ctx.enter_context(tc.tile_pool(name="pgp", bufs=2, space="PSUM"))
            pop = ctx.enter_context(tc.tile_pool(name="pop", bufs=3, space="PSUM"))

            # biases ride the idle gpsimd ring so the sync ring's first
            # issue is section 0's W1; gate weights ride the scalar ring
            # right after the first token load (see after_xg below)
            b1_sb = consts.tile([128, n_slots * 4], f32, tag="b1")
            nc.gpsimd.dma_start(b1_sb[:], b1s[:])
            b2_sb = consts.tile([128, n_slots * 8], f32, tag="b2")
            nc.gpsimd.dma_start(b2_sb[:], b2s[:])
            sb1_sb = consts.tile([128, 4], f32, tag="sb1")
            nc.gpsimd.dma_start(sb1_sb[:], sb1[:])
            sb2_sb = consts.tile([128, 8], f32, tag="sb2")
            nc.gpsimd.dma_start(sb2_sb[:], sb2[:])
            wg_sb = consts.tile([128, KH * H], wdt, tag="wg")
            swg_sb = consts.tile([128, KH * H], wdt, tag="swg")

            def section(cap, w1_src, w2_src, xg_src, out_dst, wgt, b1ap, b2ap,
                        after_xg=None, last=False, w1_tiles=None, xg_tiles=None,
                        after_loads=None):
                """One expert FFN pass over `cap` tokens (transposed layout).
                PSUM tiles hold pairs of m-tiles (one bank) so PSUM->SBUF
                moves are single fat ops when biases are zero."""
                p2 = 2 if cap <= 256 else 1   # m-tiles per PSUM bank
                if xg_tiles is None:
                    xgsb = xgp.tile([128, KD * cap], wdt, tag="xg")
                    nc.scalar.dma_start(xgsb[:], xg_src)
                if after_xg is not None:
                    after_xg()
                if w1_tiles is None:
                    w1sb = w1p.tile([128, KD * H], wdt, tag="w1")
                    nc.sync.dma_start(w1sb[:], w1_src)
                w2sb = w2p.tile([128, KH * D], wdt, tag="w2")
                nc.sync.dma_start(w2sb[:], w2_src)
                if after_loads is not None:
                    after_loads()

                def w1ap(m, k):
                    if w1_tiles is not None:
                        return w1_tiles[m][:, k * 128 : (k + 1) * 128]
                    return w1sb[:, m * KD * 128 + k * 128 : m * KD * 128 + (k + 1) * 128]

                def xgap(k):
                    if xg_tiles is not None:
                        return xg_tiles[k][:]
                    return xgsb[:, k * cap : (k + 1) * cap]

                hsb = hp.tile([128, KH * cap], wdt, tag="h")
                for mp in range(KH // p2):
                    ph = php.tile([128, p2 * cap], f32, tag="ph")
                    for sub in range(p2):
                        m = p2 * mp + sub
                        for k in range(KD):
                            nc.tensor.matmul(
                                ph[:, sub * cap : (sub + 1) * cap],
                                w1ap(m, k),
                                xgap(k),
                                start=(k == 0),
                                stop=(k == KD - 1),
                            )
                    if has_bias:
                        for sub in range(p2):
                            m = p2 * mp + sub
                            nc.vector.tensor_scalar_add(
                                hsb[:, m * cap : (m + 1) * cap],
                                ph[:, sub * cap : (sub + 1) * cap],
                                b1ap[:, m : m + 1],
                            )
                    else:
                        nc.vector.tensor_copy(
                            hsb[:, p2 * mp * cap : p2 * (mp + 1) * cap], ph[:]
                        )

                gsb = gp.tile([128, KH * cap], wdt, tag="g")
                for mp in range(KH // p2):
                    pg = pgp.tile([128, p2 * cap], f32, tag="pg")
                    for sub in range(p2):
                        m = p2 * mp + sub
                        for k in range(KH):
                            nc.tensor.matmul(
                                pg[:, sub * cap : (sub + 1) * cap],
                                wgt[:, k * H + m * 128 : k * H + (m + 1) * 128],
                                hsb[:, k * cap : (k + 1) * cap],
                                start=(k == 0),
                                stop=(k == KH - 1),
                            )
                    gpre = gpp.tile([128, p2 * cap], wdt, tag="gpre")
                    nc.scalar.activation(gpre[:], pg[:], AF.Silu)
                    nc.vector.tensor_mul(
                        gsb[:, p2 * mp * cap : p2 * (mp + 1) * cap],
                        gpre[:],
                        hsb[:, p2 * mp * cap : p2 * (mp + 1) * cap],
                    )

                osb = op.tile([128, KD * cap], odt, tag="o")
                for mp in range(KD // p2):
                    po = pop.tile([128, p2 * cap], f32, tag="po")
                    for sub in range(p2):
                        m = p2 * mp + sub
                        for k in range(KH):
                            nc.tensor.matmul(
                                po[:, sub * cap : (sub + 1) * cap],
                                w2sb[:, k * D + m * 128 : k * D + (m + 1) * 128],
                                gsb[:, k * cap : (k + 1) * cap],
                                start=(k == 0),
                                stop=(k == KH - 1),
                            )
                    if has_bias:
                        for sub in range(p2):
                            m = p2 * mp + sub
                            nc.vector.tensor_scalar_add(
                                osb[:, m * cap : (m + 1) * cap],
                                po[:, sub * cap : (sub + 1) * cap],
                                b2ap[:, m : m + 1],
                            )
                    else:
                        nc.vector.tensor_copy(
                            osb[:, p2 * mp * cap : p2 * (mp + 1) * cap], po[:]
                        )
                    if last:
                        # final section: store chunks immediately so the
                        # kernel tail is one chunk, not 2MB
                        nc.sync.dma_start(
                            out_dst[:, p2 * mp * cap : p2 * (mp + 1) * cap],
                            osb[:, p2 * mp * cap : p2 * (mp + 1) * cap],
                        )
                if last:
                    return None
                def store():
                    nc.sync.dma_start(out_dst, osb[:])
                return store

            def load_wg():
                nc.scalar.dma_start(wg_sb[:], wg[:])

            def load_swg():
                nc.scalar.dma_start(swg_sb[:], swg[:])

            # section 0 fast start: W1 as 4 independent m-tiles and tokens as
            # 8 k-tiles so the first matmul waits for ~600KB, not 3MB
            c0 = caps[0]
            sec0_xg = [consts.tile([128, c0], wdt, tag=f"xh{k}", name=f"xh{k}") for k in range(KD)]
            nc.scalar.dma_start(sec0_xg[0][:], xgt[:, 0:c0])
            nc.scalar.dma_start(sec0_xg[1][:], xgt[:, c0 : 2 * c0])
            load_wg()
            for k in range(2, KD):
                nc.scalar.dma_start(sec0_xg[k][:], xgt[:, k * c0 : (k + 1) * c0])
            sec0_w1 = [consts.tile([128, KD * 128], wdt, tag=f"w1h{m}", name=f"w1h{m}") for m in range(KH)]
            for m in range(KH):
                nc.sync.dma_start(
                    sec0_w1[m][:], w1s[0][:, m * KD * 128 : (m + 1) * KD * 128]
                )

            pending = []

            def flush_store():
                if len(pending) >= 2:
                    pending.pop(0)()

            for j in range(n_slots):
                lo, hi = int(offs[j]) * KD, int(offs[j + 1]) * KD
                st = section(
                    caps[j],
                    w1s[j],
                    w2s[j],
                    xgt[:, lo:hi],
                    rout[:, lo:hi],
                    wg_sb,
                    b1_sb[:, j * 4 : (j + 1) * 4],
                    b2_sb[:, j * 8 : (j + 1) * 8],
                    after_xg=load_swg if j == 1 else None,
                    w1_tiles=sec0_w1 if j == 0 else None,
                    xg_tiles=sec0_xg if j == 0 else None,
                    after_loads=flush_store,
                )
                pending.append(st)

            section(sh_cap, sw1[:], sw2[:], tsht[:], sout[:], swg_sb, sb1_sb,
                    sb2_sb, last=True, after_loads=flush_store)
            for st in pending:
                st()

    nc.compile()
    return nc


def _get_program(caps, has_bias):
    key = (tuple(caps), PREC, has_bias)
    if key not in _PROG_CACHE:
        _PROG_CACHE[key] = _build_program(tuple(caps), has_bias)
    return _PROG_CACHE[key]


def _route(x, norm_w, Wr, bias):
    """Host-side norm + router + top-k (matches jax.lax.top_k tie-breaking)."""
    xf = x.reshape(T, D).astype(np.float32)
    ms = np.mean(xf * xf, axis=-1, keepdims=True, dtype=np.float32)
    t = (xf * (1.0 / np.sqrt(ms + EPS)) * norm_w).astype(np.float32)
    raw = t @ Wr.T
    aff = raw + bias
    idx = np.argsort(-aff, axis=-1, kind="stable")[:, :K]
    aff_k = np.take_along_axis(raw, idx, axis=1)
    w = aff_k / aff_k.sum(-1, keepdims=True)
    return t, idx.astype(np.int64), w.astype(np.float32)


def _gather_block(t, toks, cap):
    """tokens (cnt, D) -> [128, KD, cap] SBUF layout block (zero padded)."""
    blk = np.zeros((128, KD, cap), _np_wdt())
    g = t[toks].T.reshape(KD, 128, len(toks)).transpose(1, 0, 2)
    blk[:, :, : len(toks)] = g
    return blk


def _decode_block(blk, cnt):
    """[128, KD, cap] device output block -> (cnt, D) token outputs."""
    cap = blk.shape[2]
    return blk.transpose(1, 0, 2).reshape(D, cap)[:, :cnt].T.astype(np.float32)


def kernel(**inputs):
    x = np.asarray(inputs["x"], dtype=np.float32)
    norm_w = np.asarray(inputs["norm_w"], dtype=np.float32)
    Wr = np.asarray(inputs["Wr"], dtype=np.float32)
    bias = np.asarray(inputs["bias"], dtype=np.float32)
    sW1 = np.asarray(inputs["sW1"], dtype=np.float32)
    sb1 = np.asarray(inputs["sb1"], dtype=np.float32)
    sW2 = np.asarray(inputs["sW2"], dtype=np.float32)
    sb2 = np.asarray(inputs["sb2"], dtype=np.float32)
    sWg = np.asarray(inputs["sWg"], dtype=np.float32)
    rW1 = np.asarray(inputs["rW1"], dtype=np.float32)
    rb1 = np.asarray(inputs["rb1"], dtype=np.float32)
    rW2 = np.asarray(inputs["rW2"], dtype=np.float32)
    rb2 = np.asarray(inputs["rb2"], dtype=np.float32)
    rWg = np.asarray(inputs["rWg"], dtype=np.float32)

    t, idx, w = _route(x, norm_w, Wr, bias)

    # per-expert token lists (token order ascending within each expert)
    flat_e = idx.ravel()
    flat_tok = np.repeat(np.arange(T), K)
    flat_w = w.ravel()
    order = np.argsort(flat_e, kind="stable")
    se, st, sw = flat_e[order], flat_tok[order], flat_w[order]
    counts = np.bincount(flat_e, minlength=E_R)
    bounds = np.concatenate([[0], np.cumsum(counts)])

    # split any over-512 expert into <=512 pieces (512 = max matmul free dim
    # for one PSUM bank at fp32)
    pieces = []  # (expert, tok_ids, weights)
    for e in range(E_R):
        lo, hi = bounds[e], bounds[e + 1]
        for s in range(lo, hi, 512):
            pieces.append((e, st[s : min(s + 512, hi)], sw[s : min(s + 512, hi)]))
    n_slots = max(N_SLOTS, -(-len(pieces) // 8))

    # snake assignment: sort pieces by size desc; rank-group of 8 -> one slot
    # index across all cores; within each group assign large->small to the
    # cores with the smallest running totals. Slot capacity = group max
    # rounded up to 16 (compile-time constant; identical inputs -> identical
    # caps -> NEFF cache hit).
    pieces.sort(key=lambda p: -len(p[1]))
    slot_of_core = [[None] * n_slots for _ in range(8)]
    totals = np.zeros(8, np.int64)
    caps = []
    for j in range(n_slots):
        group = pieces[j * 8 : (j + 1) * 8]
        core_order = np.argsort(totals, kind="stable")
        for gi, piece in enumerate(group):
            c = core_order[gi]
            slot_of_core[c][j] = piece
            totals[c] += len(piece[1])
        gmax = max((len(p[1]) for p in group), default=16)
        caps.append(min(512, max(32, -(-gmax // 16) * 16)))
    if PREC == "f32r":
        caps = [max(256, c) for c in caps]  # f32r needs N>=256 for full rate
    caps.append(SH_TOK)

    has_bias = bool(
        np.any(rb1) or np.any(rb2) or np.any(sb1) or np.any(sb2)
    )
    nc = _get_program(caps, has_bias)
    offs = np.concatenate([[0], np.cumsum(caps)]).astype(int)
    sumcap = int(offs[-2])

    wg_pre = _prearrange(rWg, KH)
    swg_pre = _prearrange(sWg, KH)
    sw1_pre = [_prearrange_w1(sW1[e]) for e in range(E_S)]
    sw2_pre = [_prearrange(sW2[e], KH) for e in range(E_S)]
    w1_pre = {}
    w2_pre = {}
    in_maps = []
    for c in range(8):
        xgt = np.zeros((128, KD * sumcap), _np_wdt())
        w1_stack = np.zeros((n_slots, 128, KD * H), _np_wdt())
        w2_stack = np.zeros((n_slots, 128, KH * D), _np_wdt())
        b1_arr = np.zeros((128, n_slots * 4), np.float32)
        b2_arr = np.zeros((128, n_slots * 8), np.float32)
        for j in range(n_slots):
            piece = slot_of_core[c][j]
            if piece is None:
                continue
            e, toks, _ = piece
            xgt[:, offs[j] * KD : offs[j + 1] * KD] = _gather_block(
                t, toks, caps[j]
            ).reshape(128, KD * caps[j])
            if e not in w1_pre:
                w1_pre[e] = _prearrange_w1(rW1[e])
                w2_pre[e] = _prearrange(rW2[e], KH)
            w1_stack[j] = w1_pre[e]
            w2_stack[j] = w2_pre[e]
            b1_arr[:, j * 4 : (j + 1) * 4] = rb1[e, 0].reshape(4, 128).T
            b2_arr[:, j * 8 : (j + 1) * 8] = rb2[e, 0].reshape(8, 128).T
        qc, se_ = c % 4, c // 4
        sh_toks = np.arange(qc * SH_TOK, (qc + 1) * SH_TOK)
        in_maps.append({
            "xgt": xgt,
            "w1s": w1_stack,
            "w2s": w2_stack,
            "b1s": b1_arr,
            "b2s": b2_arr,
            "wg": wg_pre,
            "swg": swg_pre,
            "tsht": _gather_block(t, sh_toks, SH_TOK).reshape(128, KD * SH_TOK),
            "sw1": sw1_pre[se_],
            "sw2": sw2_pre[se_],
            "sb1": sb1[se_, 0].reshape(4, 128).T.copy(),
            "sb2": sb2[se_, 0].reshape(8, 128).T.copy(),
        })

    from concourse.bass_utils import run_bass_kernel_spmd

    global _LAST_IN_MAPS
    _LAST_IN_MAPS = in_maps
    res = run_bass_kernel_spmd(nc, in_maps, core_ids=list(range(8)))

    out = x.reshape(T, D).copy()
    for c in range(8):
        qc = c % 4
        so = res.results[c]["sout"].reshape(128, KD, SH_TOK)
        out[qc * SH_TOK : (qc + 1) * SH_TOK] += _decode_block(so, SH_TOK)
        ro = res.results[c]["rout"]
        for j in range(n_slots):
            piece = slot_of_core[c][j]
            if piece is None:
                continue
            _, toks, wv = piece
            blk = ro[:, offs[j] * KD : offs[j + 1] * KD].reshape(128, KD, caps[j])
            out[toks] += wv[:, None] * _decode_block(blk, len(toks))
    return out.reshape(B, S, D).astype(np.float32)
